# revision 13
# baseline (speedup 1.0000x reference)
"""Trainium2 Bass kernel for DeterministicActorRNN.

Network (per sample):
  obs   = state[:, :1280] -> 5 frames of 256, time-reversed
  2-layer tanh RNN (H=256, T=5)  -> last hidden of layer 2
  MLP: relu(320->1024), relu(1024->1024), 64 <- tanh(1024->64)

Strategy: pure data parallel over 8 NeuronCores (2048 rows each).
All activations are kept feature-major ([feature, batch]) so every matmul
contracts the partition dimension; the host pre-transposes the state slice
and all weight matrices.  Matmuls run in float32r (fp32 storage, 11-bit
mantissa multiply) at full PE rate; PSUM accumulation and all bias/tanh/relu
math stay fp32.  Each core processes its 2048 rows as 2 independent
batch-chunks of 1024 to bound SBUF usage and give the scheduler two
independent RNN chains to interleave.
"""

import numpy as np

B = 16384
NCORES = 8
BL = B // NCORES          # 2048 rows per core
NPASS = 2
BP = BL // NPASS          # 1024 rows per pass
T = 5
H = 256                   # RNN hidden / obs per frame
G = 64                    # goal dim
OBS = T * H               # 1280
HID = 1024
OUT = 64
NKH = H // 128            # 2 feature blocks of 128

_EXEC = None              # compiled executable cache


def _build_bass():
    import os
    import concourse.bass as bass  # noqa: F401
    from concourse import bacc
    import concourse.mybir as mybir
    from concourse.tile import TileContext

    K_T = int(os.environ.get("K_T", T))          # RNN steps to run
    K_MLP = int(os.environ.get("K_MLP", "1"))    # include MLP
    K_RNN = int(os.environ.get("K_RNN", "1"))    # include RNN

    F32 = mybir.dt.float32
    F32R = mybir.dt.float32r
    TANH = mybir.ActivationFunctionType.Tanh
    RELU = mybir.ActivationFunctionType.Relu

    nc = bacc.Bacc(None, target_bir_lowering=False)

    xT = nc.dram_tensor("xT", [OBS + G, BL], F32R, kind="ExternalInput")
    wi_d = [nc.dram_tensor(f"wi{l}T", [H, H], F32R, kind="ExternalInput") for l in range(2)]
    wh_d = [nc.dram_tensor(f"wh{l}T", [H, H], F32R, kind="ExternalInput") for l in range(2)]
    brnn_d = [nc.dram_tensor(f"brnn{l}", [H], F32, kind="ExternalInput") for l in range(2)]
    w1_d = nc.dram_tensor("w1T", [H + G, HID], F32R, kind="ExternalInput")
    w2_d = nc.dram_tensor("w2T", [HID, HID], F32R, kind="ExternalInput")
    w3_d = nc.dram_tensor("w3T", [HID, OUT], F32R, kind="ExternalInput")
    b1_d = nc.dram_tensor("b1v", [HID], F32, kind="ExternalInput")
    b2_d = nc.dram_tensor("b2v", [HID], F32, kind="ExternalInput")
    b3_d = nc.dram_tensor("b3v", [OUT, 1], F32, kind="ExternalInput")
    out_d = nc.dram_tensor("out", [OUT, BL], F32, kind="ExternalOutput")

    with TileContext(nc) as tc:
        with (
            tc.tile_pool(name="wp", bufs=1) as wp,
            tc.tile_pool(name="xp", bufs=6) as xp,
            tc.tile_pool(name="hp", bufs=10) as hp,
            tc.tile_pool(name="mp", bufs=12) as mp,
            tc.tile_pool(name="op", bufs=1) as op,
            tc.tile_pool(name="ps", bufs=3, space="PSUM") as psp,
            tc.tile_pool(name="ps3", bufs=1, space="PSUM") as ps3p,
        ):
            # ---- RNN weights + biases ----
            w_rnn = {}
            for l in range(2):
                for key, drt in (("i", wi_d[l]), ("h", wh_d[l])):
                    tw = wp.tile([128, NKH, H], F32R, tag=f"w{key}{l}")
                    nc.sync.dma_start(
                        out=tw, in_=drt[:, :].rearrange("(ko ki) m -> ki ko m", ki=128)
                    )
                    w_rnn[(l, key)] = tw
            b_rnn = []
            for l in range(2):
                tb = wp.tile([128, NKH], F32, tag=f"brnn{l}")
                nc.sync.dma_start(
                    out=tb, in_=brnn_d[l][:].rearrange("(ko ki) -> ki ko", ki=128)
                )
                b_rnn.append(tb)

            # ---- obs tiles for t=0 and goal (critical path first) ----
            x_tiles = {}  # (p, t, kb) -> tile [128, BP]

            def load_x(p, t, kb):
                fr = (T - 1) - t  # time reversal
                tile = xp.tile([128, BP], F32R, tag="x")
                r0 = fr * H + kb * 128
                nc.sync.dma_start(
                    out=tile, in_=xT[r0 : r0 + 128, p * BP : (p + 1) * BP]
                )
                x_tiles[(p, t, kb)] = tile

            for p in range(NPASS):
                for kb in range(NKH):
                    load_x(p, 0, kb)
            goal_sb = op.tile([G, BL], F32R, tag="goal")
            nc.sync.dma_start(out=goal_sb, in_=xT[OBS : OBS + G, :])
            for t in range(1, T):
                for p in range(NPASS):
                    for kb in range(NKH):
                        load_x(p, t, kb)

            # ---- MLP weights + biases ----
            w1_sb = wp.tile([128, 3, HID], F32R, tag="w1")
            nc.sync.dma_start(
                out=w1_sb[:, 0:2, :],
                in_=w1_d[0:H, :].rearrange("(ko ki) m -> ki ko m", ki=128),
            )
            nc.sync.dma_start(out=w1_sb[:G, 2, :], in_=w1_d[H : H + G, :])
            w2_sb = wp.tile([128, 8, HID], F32R, tag="w2")
            nc.sync.dma_start(
                out=w2_sb, in_=w2_d[:, :].rearrange("(ko ki) m -> ki ko m", ki=128)
            )
            w3_sb = wp.tile([128, 8, OUT], F32R, tag="w3")
            nc.sync.dma_start(
                out=w3_sb, in_=w3_d[:, :].rearrange("(ko ki) m -> ki ko m", ki=128)
            )
            b1_sb = wp.tile([128, 8], F32, tag="b1")
            nc.sync.dma_start(out=b1_sb, in_=b1_d[:].rearrange("(ko ki) -> ki ko", ki=128))
            b2_sb = wp.tile([128, 8], F32, tag="b2")
            nc.sync.dma_start(out=b2_sb, in_=b2_d[:].rearrange("(ko ki) -> ki ko", ki=128))
            b3_sb = wp.tile([G, 1], F32, tag="b3")
            nc.sync.dma_start(out=b3_sb, in_=b3_d[:, :])

            # ---- RNN: 2 layers x 5 steps, both passes interleaved ----
            h_prev = {p: [None, None] for p in range(NPASS)}  # per layer: [m0, m1]
            h_cur = {p: [None, None] for p in range(NPASS)}
            for t in range(K_T if K_RNN else 0):
                for l in range(2):
                    for p in range(NPASS):
                        if l == 0:
                            src = [x_tiles[(p, t, kb)] for kb in range(NKH)]
                        else:
                            src = h_cur[p][0]
                        prev = h_prev[p][l]
                        new = []
                        for m in range(NKH):
                            ps = psp.tile([128, BP], F32, tag="ps")
                            msl = slice(m * 128, (m + 1) * 128)
                            for n in range(BP // 512):
                                nsl = slice(n * 512, (n + 1) * 512)
                                ops = [
                                    (src[kb], w_rnn[(l, "i")][:, kb, msl])
                                    for kb in range(NKH)
                                ]
                                if prev is not None:
                                    ops += [
                                        (prev[kb], w_rnn[(l, "h")][:, kb, msl])
                                        for kb in range(NKH)
                                    ]
                                for i, (rhs_t, w_ap) in enumerate(ops):
                                    nc.tensor.matmul(
                                        ps[:, nsl],
                                        w_ap,
                                        rhs_t[:, nsl],
                                        start=(i == 0),
                                        stop=(i == len(ops) - 1),
                                    )
                            ht = hp.tile([128, BP], F32R, tag="h")
                            nc.scalar.activation(
                                ht[:, :], ps[:, :], TANH, bias=b_rnn[l][:, m : m + 1]
                            )
                            new.append(ht)
                        h_cur[p][l] = new
                for p in range(NPASS):
                    h_prev[p] = [h_cur[p][0], h_cur[p][1]]

            # ---- MLP ----
            out_sb = op.tile([OUT, BL], F32, tag="o")
            if not K_MLP:
                nc.vector.memset(out_sb[:, :], 0.0)
            for p in range(NPASS if K_MLP else 0):
                h2last = h_cur[p][1]
                if h2last is None:  # RNN skipped: use raw x tiles instead
                    h2last = [x_tiles[(p, 0, kb)] for kb in range(NKH)]
                # layer 1: [320 -> 1024]
                h1_tiles = []
                for m in range(8):
                    ps = psp.tile([128, BP], F32, tag="ps")
                    msl = slice(m * 128, (m + 1) * 128)
                    for n in range(BP // 512):
                        nsl = slice(n * 512, (n + 1) * 512)
                        gsl = slice(p * BP + n * 512, p * BP + (n + 1) * 512)
                        ops = [
                            (h2last[kb][:, nsl], w1_sb[:, kb, msl]) for kb in range(NKH)
                        ] + [(goal_sb[:, gsl], w1_sb[:G, 2, msl])]
                        for i, (rhs_ap, w_ap) in enumerate(ops):
                            nc.tensor.matmul(
                                ps[:, nsl],
                                w_ap,
                                rhs_ap,
                                start=(i == 0),
                                stop=(i == len(ops) - 1),
                            )
                    t1 = mp.tile([128, BP], F32R, tag="m")
                    nc.scalar.activation(
                        t1[:, :], ps[:, :], RELU, bias=b1_sb[:, m : m + 1]
                    )
                    h1_tiles.append(t1)
                # layer 2: [1024 -> 1024], layer 3 accumulated incrementally
                ps3 = ps3p.tile([G, BP], F32, tag="ps3")
                for m in range(8):
                    ps = psp.tile([128, BP], F32, tag="ps")
                    msl = slice(m * 128, (m + 1) * 128)
                    for n in range(BP // 512):
                        nsl = slice(n * 512, (n + 1) * 512)
                        for k in range(8):
                            nc.tensor.matmul(
                                ps[:, nsl],
                                w2_sb[:, k, msl],
                                h1_tiles[k][:, nsl],
                                start=(k == 0),
                                stop=(k == 7),
                            )
                    t2 = mp.tile([128, BP], F32R, tag="m")
                    nc.scalar.activation(
                        t2[:, :], ps[:, :], RELU, bias=b2_sb[:, m : m + 1]
                    )
                    for n in range(BP // 512):
                        nsl = slice(n * 512, (n + 1) * 512)
                        nc.tensor.matmul(
                            ps3[:, nsl],
                            w3_sb[:, m, :],
                            t2[:, nsl],
                            start=(m == 0),
                            stop=(m == 7),
                        )
                nc.scalar.activation(
                    out_sb[:, p * BP : (p + 1) * BP], ps3[:, :], TANH, bias=b3_sb[:, 0:1]
                )
            nc.sync.dma_start(out=out_d[:, :], in_=out_sb)

    nc.finalize()
    return nc


class _Executor:
    """Compile a bass module once; run it on 8 cores via the axon PJRT path."""

    def __init__(self, build_fn=None):
        import jax
        from concourse import bass2jax
        import concourse.mybir as mybir

        self.jax = jax
        self.bass2jax = bass2jax
        bass2jax.install_neuronx_cc_hook()
        nc = (build_fn or _build_bass)()
        self.nc = nc

        self.partition_name = (
            nc.partition_id_tensor.name if nc.partition_id_tensor else None
        )
        in_names, out_names, out_avals = [], [], []
        for alloc in nc.m.functions[0].allocations:
            if not isinstance(alloc, mybir.MemoryLocationSet):
                continue
            name = alloc.memorylocations[0].name
            if alloc.kind == "ExternalInput":
                if name != self.partition_name:
                    in_names.append(name)
            elif alloc.kind == "ExternalOutput":
                out_names.append(name)
                out_avals.append(
                    jax.core.ShapedArray(tuple(alloc.tensor_shape), mybir.dt.np(alloc.dtype))
                )
        self.in_names = in_names
        self.out_names = out_names
        self.out_avals = out_avals
        self._jitted = {}

    def _make(self, repeat):
        import jax
        from jax.experimental.shard_map import shard_map
        from jax.sharding import Mesh, PartitionSpec
        import numpy as np

        n_in = len(self.in_names)
        n_out = len(self.out_names)
        all_names = tuple(self.in_names) + tuple(self.out_names)
        if self.partition_name is not None:
            all_names = all_names + (self.partition_name,)
        nc = self.nc
        out_avals = tuple(self.out_avals)
        bass2jax = self.bass2jax

        def _body(*args):
            ins = list(args[:n_in])
            outs = list(args[n_in:])
            extra = (
                [bass2jax.partition_id_tensor()]
                if self.partition_name is not None
                else []
            )
            for _ in range(repeat):
                outs = list(
                    bass2jax._bass_exec_p.bind(
                        *ins,
                        *outs,
                        *extra,
                        out_avals=out_avals,
                        in_names=all_names,
                        out_names=tuple(self.out_names),
                        lowering_input_output_aliases=(),
                        sim_require_finite=True,
                        sim_require_nnan=True,
                        nc=nc,
                    )
                )
            return tuple(outs)

        devices = jax.devices()[:NCORES]
        mesh = Mesh(np.asarray(devices), ("core",))
        specs = (PartitionSpec("core"),) * (n_in + n_out)
        out_specs = (PartitionSpec("core"),) * n_out
        donate = tuple(range(n_in, n_in + n_out))
        return jax.jit(
            shard_map(_body, mesh=mesh, in_specs=specs, out_specs=out_specs,
                      check_rep=False),
            donate_argnums=donate,
            keep_unused=True,
        )

    def jitted(self, repeat=1):
        if repeat not in self._jitted:
            self._jitted[repeat] = self._make(repeat)
        return self._jitted[repeat]

    def concat_inputs(self, in_maps):
        return [
            np.concatenate([np.asarray(m[name]) for m in in_maps], axis=0)
            for name in self.in_names
        ]

    def zero_outs(self):
        return [
            np.zeros((NCORES * a.shape[0], *a.shape[1:]), a.dtype)
            for a in self.out_avals
        ]

    def run(self, in_maps, repeat=1):
        fn = self.jitted(repeat)
        outs = fn(*self.concat_inputs(in_maps), *self.zero_outs())
        res = []
        for c in range(NCORES):
            res.append(
                {
                    name: np.asarray(outs[i]).reshape(NCORES, *self.out_avals[i].shape)[c]
                    for i, name in enumerate(self.out_names)
                }
            )
        return res


def _get_exec():
    global _EXEC
    if _EXEC is None:
        _EXEC = _Executor()
    return _EXEC


def _prep_inputs(state, rnn_Wih, rnn_Whh, rnn_bih, rnn_bhh, W1, b1, W2, b2, W3, b3):
    f32 = np.float32
    state = np.asarray(state, f32)
    shared = {
        "wi0T": np.ascontiguousarray(np.asarray(rnn_Wih[0], f32).T),
        "wh0T": np.ascontiguousarray(np.asarray(rnn_Whh[0], f32).T),
        "wi1T": np.ascontiguousarray(np.asarray(rnn_Wih[1], f32).T),
        "wh1T": np.ascontiguousarray(np.asarray(rnn_Whh[1], f32).T),
        "brnn0": np.asarray(rnn_bih[0], f32) + np.asarray(rnn_bhh[0], f32),
        "brnn1": np.asarray(rnn_bih[1], f32) + np.asarray(rnn_bhh[1], f32),
        "w1T": np.ascontiguousarray(np.asarray(W1, f32).T),
        "w2T": np.ascontiguousarray(np.asarray(W2, f32).T),
        "w3T": np.ascontiguousarray(np.asarray(W3, f32).T),
        "b1v": np.asarray(b1, f32),
        "b2v": np.asarray(b2, f32),
        "b3v": np.asarray(b3, f32).reshape(OUT, 1),
    }
    in_maps = []
    for c in range(NCORES):
        xT = np.ascontiguousarray(state[c * BL : (c + 1) * BL, :].T)
        in_maps.append({"xT": xT, **shared})
    return in_maps


def kernel(state, rnn_Wih, rnn_Whh, rnn_bih, rnn_bhh, W1, b1, W2, b2, W3, b3):
    ex = _get_exec()
    in_maps = _prep_inputs(
        state, rnn_Wih, rnn_Whh, rnn_bih, rnn_bhh, W1, b1, W2, b2, W3, b3
    )
    res = ex.run(in_maps)
    return np.concatenate([res[c]["out"].T for c in range(NCORES)], axis=0)
